# revision 4
# baseline (speedup 1.0000x reference)
"""Trainium2 Bass kernel for DiamondLayer.

Computes out[b, d] = mean(x[b, d:d+16, d+17:d+33]) for d in [0, 2016):
16x16 mean-pool windows sliding along the diagonal of each 2048x2048 matrix.

Sharding: pure data parallel over batch - 32 batches -> 8 cores x 4 batches.

Per-core kernel (raw bacc, no Tile). Partition p holds band rows
[16p, 16p+16), each row the 32 cols [r+2, r+34) (one 128B DMA run per
row; ~1/64 of the matrix). Per batch:

  band DMA (sync queue, 127 partitions x 16 rows x 128B)
  -> DVE prefix scan P over the flat band
  -> GpSimd sub C[16t+m] = P[32t+m+16] - P[32t+m]   (contiguous 256)
  -> S16 halo (scalar queue): C[q+1, 0:240] -> C[q, 256:496], so
     partition q holds window sums for all 31 rows its diamonds touch
     (C[16tau+m], tau in [0,31), is injective: 16u+15s collides only at
     |u-u'| = 15 which is out of range)
  -> DVE reduce out[16q+u] = sum_s C[q, 15+16u+15s]  (256 reads)
  -> ACT scale 1/256 -> out DMA (scalar queue).

vs the previous version: the halo moves 960B/partition of window sums
instead of 1924B of prefix values, there is no separate subtract of
halo'd prefixes, and band DMAs carry all 127 partitions in one shot.
Per-DMA semaphores; compute deps via counting sems.
"""

import os
import sys

import numpy as np

for _p in ("/opt/trn_rl_repo",):
    if _p not in sys.path:
        sys.path.insert(0, _p)

B_FULL = 32
N_CORES = 8
B_PER_CORE = B_FULL // N_CORES  # 4
MAT = 2048
ND = MAT - 32  # 2016
NQ = ND // 16  # 126  (diamond groups)
NP = NQ + 1  # 127  (partitions holding band rows)
ROW_STRIDE = MAT + 1  # 2049
MAT_ELEMS = MAT * MAT
PW = 528  # band / prefix cols per partition (needs 513)
CW = 512  # window-sum cols per partition (uses 496)

LAST_EXEC_TIME_NS = None
LAST_TRACE_DIR = None
_COMPILED = None


def _build():
    import concourse.bass as bass
    import concourse.bacc as bacc
    from concourse import mybir
    from contextlib import ExitStack

    f32 = mybir.dt.float32
    add = mybir.AluOpType.add
    sub_op = mybir.AluOpType.subtract
    bypass = mybir.AluOpType.bypass
    X = mybir.AxisListType.X

    nc = bacc.Bacc("TRN2", target_bir_lowering=False, debug=False)
    x = nc.dram_tensor("x", [B_PER_CORE, MAT, MAT], f32, kind="ExternalInput")
    y = nc.dram_tensor("y", [B_PER_CORE, ND], f32, kind="ExternalOutput")

    def v(t, off, pat):
        return bass.AP(t, off, pat)

    with ExitStack() as ctx:
        B = B_PER_CORE
        e = ctx.enter_context
        bts = [e(nc.sbuf_tensor(f"bt{i}", [NP, PW], f32)) for i in range(B)]
        pps = [e(nc.sbuf_tensor(f"pp{i}", [NP, PW], f32)) for i in range(B)]
        cs = [e(nc.sbuf_tensor(f"c{i}", [NP, CW], f32)) for i in range(B)]
        yvs = [e(nc.sbuf_tensor(f"yv{i}", [NQ, 16], f32)) for i in range(B)]
        yos = [e(nc.sbuf_tensor(f"yo{i}", [NQ, 16], f32)) for i in range(B)]
        bsem = [e(nc.semaphore(f"bsem{i}")) for i in range(B)]
        hsem = [e(nc.semaphore(f"hsem{i}")) for i in range(B)]
        vscan = e(nc.semaphore("vscan"))
        initsem = e(nc.semaphore("initsem"))
        subsem = e(nc.semaphore("subsem"))
        redsem = e(nc.semaphore("redsem"))
        scalesem = e(nc.semaphore("scalesem"))
        outsem = e(nc.semaphore("outsem"))
        block = e(nc.Block(no_gpsimd_drain=True))

        @block.sync
        def _(sync):
            for b in range(B):
                # band: bt[p, 1+32t+j] = x[b, 16p+t, 16p+t+2+j]
                sync.dma_start(
                    v(bts[b], 1, [[PW, NP], [32, 16], [1, 32]]),
                    bass.AP(
                        x,
                        b * MAT_ELEMS + 2,
                        [[16 * ROW_STRIDE, NP], [ROW_STRIDE, 16], [1, 32]],
                    ),
                ).then_inc(bsem[b], 16)
            sync.wait_ge(outsem, 16 * B)

        @block.vector
        def _(vector):
            def scan(b):
                vector.wait_ge(bsem[b], 16)
                # P[f] = prefix sum of the flat band per partition; P[0] = 0
                nc.vector.tensor_tensor_scan(
                    out=v(pps[b], 1, [[PW, NP], [1, 511]]),
                    data0=v(bts[b], 1, [[PW, NP], [1, 511]]),
                    data1=v(bts[b], 1, [[PW, NP], [1, 511]]),
                    initial=0.0,
                    op0=add,
                    op1=bypass,
                ).then_inc(vscan, 1)

            def red(b):
                # out[16q+u] = sum_s C[q, 15+16u+15s]
                vector.wait_ge(hsem[b], 16)
                nc.vector.reduce_sum(
                    out=v(yvs[b], 0, [[16, NQ], [1, 16]]),
                    in_=v(cs[b], 15, [[CW, NQ], [16, 16], [15, 16]]),
                    axis=X,
                ).then_inc(redsem, 1)

            for b in range(B):
                m = nc.vector.memset(pps[b][0:NP, 0:1], 0.0)
                if b == B - 1:
                    m.then_inc(initsem, 1)
            scan(0)
            scan(1)
            scan(2)
            red(0)
            scan(3)
            red(1)
            red(2)
            red(3)

        @block.gpsimd
        def _(gpsimd):
            for b in range(B):
                # C[16t+m] = P[32t+m+16] - P[32t+m]  (local window sums)
                gpsimd.wait_ge(vscan, b + 1)
                if b == 0:
                    gpsimd.wait_ge(initsem, 1)
                nc.gpsimd.tensor_tensor(
                    out=v(cs[b], 0, [[CW, NP], [16, 16], [1, 16]]),
                    in0=v(pps[b], 16, [[PW, NP], [32, 16], [1, 16]]),
                    in1=v(pps[b], 0, [[PW, NP], [32, 16], [1, 16]]),
                    op=sub_op,
                ).then_inc(subsem, 1)

        @block.scalar
        def _(scalar):
            def halo(b):
                # C[q, 256+f] = C[q+1, f], f in [0, 240)
                scalar.wait_ge(subsem, b + 1)
                scalar.dma_start(
                    v(cs[b], 256, [[CW, NQ], [1, 240]]),
                    v(cs[b], CW, [[CW, NQ], [1, 240]]),
                ).then_inc(hsem[b], 16)

            def mulout(b):
                scalar.wait_ge(redsem, b + 1)
                nc.scalar.mul(
                    v(yos[b], 0, [[16, NQ], [1, 16]]),
                    v(yvs[b], 0, [[16, NQ], [1, 16]]),
                    1.0 / 256.0,
                ).then_inc(scalesem, 1)
                scalar.wait_ge(scalesem, b + 1)
                scalar.dma_start(
                    bass.AP(y, b * ND, [[16, NQ], [1, 16]]),
                    v(yos[b], 0, [[16, NQ], [1, 16]]),
                ).then_inc(outsem, 16)

            halo(0)
            halo(1)
            mulout(0)
            halo(2)
            mulout(1)
            halo(3)
            mulout(2)
            mulout(3)

    nc.compile()
    return nc


def _get_compiled():
    global _COMPILED
    if _COMPILED is None:
        _COMPILED = _build()
    return _COMPILED


def kernel(x: np.ndarray) -> np.ndarray:
    global LAST_EXEC_TIME_NS, LAST_TRACE_DIR
    from concourse.bass_utils import run_bass_kernel_spmd

    x = np.ascontiguousarray(np.asarray(x), dtype=np.float32)
    assert x.shape == (B_FULL, MAT, MAT), x.shape

    nc = _get_compiled()
    in_maps = [
        {"x": x[i * B_PER_CORE : (i + 1) * B_PER_CORE]} for i in range(N_CORES)
    ]
    trace = bool(int(os.environ.get("KERNEL_TRACE", "0")))
    kwargs = {}
    if trace:
        # test-only: keep NTFF artifacts local instead of uploading
        from concourse import bass_utils as _bu
        import tempfile

        _bu.upload_artifacts = lambda tmpdir: tmpdir
        LAST_TRACE_DIR = tempfile.mkdtemp(prefix="ktrace_")
        kwargs["tmpdir"] = LAST_TRACE_DIR
    res = run_bass_kernel_spmd(
        nc, in_maps, core_ids=list(range(N_CORES)), trace=trace, **kwargs
    )
    LAST_EXEC_TIME_NS = res.exec_time_ns
    out = np.concatenate([res.results[i]["y"] for i in range(N_CORES)], axis=0)
    return out.astype(np.float32)


# revision 8
# speedup vs baseline: 3.4811x; 3.4811x over previous
"""Trainium2 Bass kernel for DiamondLayer.

Computes out[b, d] = mean(x[b, d:d+16, d+17:d+33]) for d in [0, 2016):
16x16 mean-pool windows sliding along the diagonal of each 2048x2048 matrix.

Sharding: pure data parallel over batch - 32 batches -> 8 cores x 4 batches.

Per-core kernel (raw bacc, no Tile). Partition p holds band rows
[16p, 16p+16), each row the 32 cols [r+2, r+34) (one 128B DMA run per
row; ~1/64 of the matrix). Per batch:

  band DMA (sync queue, 127 partitions x 16 rows x 128B)
  -> DVE prefix scan P over the flat band
  -> GpSimd sub C[16t+m] = P[32t+m+16] - P[32t+m]   (contiguous 256)
  -> S16 halo (scalar queue): C[q+1, 0:240] -> C[q, 256:496], so
     partition q holds window sums for all 31 rows its diamonds touch
     (C[16tau+m], tau in [0,31), is injective: 16u+15s collides only at
     |u-u'| = 15 which is out of range)
  -> DVE reduce out[16q+u] = sum_s C[q, 15+16u+15s]  (256 reads)
  -> ACT scale 1/256 -> out DMA (scalar queue).

vs the previous version: the halo moves 960B/partition of window sums
instead of 1924B of prefix values, there is no separate subtract of
halo'd prefixes, and band DMAs carry all 127 partitions in one shot.
Per-DMA semaphores; compute deps via counting sems.
"""

import os
import sys

import numpy as np

for _p in ("/opt/trn_rl_repo",):
    if _p not in sys.path:
        sys.path.insert(0, _p)

B_FULL = 32
N_CORES = 8
B_PER_CORE = B_FULL // N_CORES  # 4
MAT = 2048
ND = MAT - 32  # 2016
NQ = ND // 16  # 126  (diamond groups)
NP = NQ + 1  # 127  (partitions holding band rows)
ROW_STRIDE = MAT + 1  # 2049
MAT_ELEMS = MAT * MAT
PW = 528  # band / prefix cols per partition (needs 513)
CW = 512  # window-sum cols per partition (uses 496)

LAST_EXEC_TIME_NS = None
LAST_TRACE_DIR = None
_COMPILED = None


def _build():
    import concourse.bass as bass
    import concourse.bacc as bacc
    from concourse import mybir
    from contextlib import ExitStack

    f32 = mybir.dt.float32
    add = mybir.AluOpType.add
    sub_op = mybir.AluOpType.subtract
    bypass = mybir.AluOpType.bypass
    X = mybir.AxisListType.X

    nc = bacc.Bacc("TRN2", target_bir_lowering=False, debug=False)
    x = nc.dram_tensor("x", [B_PER_CORE, MAT, MAT], f32, kind="ExternalInput")
    y = nc.dram_tensor("y", [B_PER_CORE, ND], f32, kind="ExternalOutput")

    def v(t, off, pat):
        return bass.AP(t, off, pat)

    with ExitStack() as ctx:
        B = B_PER_CORE
        e = ctx.enter_context
        bts = [e(nc.sbuf_tensor(f"bt{i}", [NP, PW], f32)) for i in range(B)]
        pps = [e(nc.sbuf_tensor(f"pp{i}", [NP, PW], f32)) for i in range(B)]
        cs = [e(nc.sbuf_tensor(f"c{i}", [NP, CW], f32)) for i in range(B)]
        yvs = [e(nc.sbuf_tensor(f"yv{i}", [NQ, 16], f32)) for i in range(B)]
        yos = [e(nc.sbuf_tensor(f"yo{i}", [NQ, 16], f32)) for i in range(B)]
        bsem = [e(nc.semaphore(f"bsem{i}")) for i in range(B)]
        tsem = [e(nc.semaphore(f"tsem{i}")) for i in range(B)]
        hsem = [e(nc.semaphore(f"hsem{i}")) for i in range(B)]
        vscan = e(nc.semaphore("vscan"))
        initsem = e(nc.semaphore("initsem"))
        subsem = e(nc.semaphore("subsem"))
        redsem = e(nc.semaphore("redsem"))
        scalesem = e(nc.semaphore("scalesem"))
        outsem = e(nc.semaphore("outsem"))
        block = e(nc.Block(no_gpsimd_drain=True))

        @block.sync
        def _(sync):
            for b in range(B):
                # band: bt[p, 1+32t+j] = x[b, 16p+t, 16p+t+2+j]
                # 126 partitions; 127 in one DMA lands all packets on a
                # single SDMA engine (odd partition count breaks the
                # HWDGE 16-way split) - tail partition goes separately.
                sync.dma_start(
                    v(bts[b], 1, [[PW, NQ], [32, 16], [1, 32]]),
                    bass.AP(
                        x,
                        b * MAT_ELEMS + 2,
                        [[16 * ROW_STRIDE, NQ], [ROW_STRIDE, 16], [1, 32]],
                    ),
                ).then_inc(bsem[b], 16)
            sync.wait_ge(outsem, 16 * B)

        @block.vector
        def _(vector):
            def scan(b):
                vector.wait_ge(bsem[b], 16)
                vector.wait_ge(tsem[b], 16)
                # P[f] = prefix sum of the flat band per partition; P[0] = 0
                nc.vector.tensor_tensor_scan(
                    out=v(pps[b], 1, [[PW, NP], [1, 511]]),
                    data0=v(bts[b], 1, [[PW, NP], [1, 511]]),
                    data1=v(bts[b], 1, [[PW, NP], [1, 511]]),
                    initial=0.0,
                    op0=add,
                    op1=bypass,
                ).then_inc(vscan, 1)

            def red(b):
                # out[16q+u] = sum_s C[q, 15+16u+15s]
                vector.wait_ge(hsem[b], 16)
                nc.vector.reduce_sum(
                    out=v(yvs[b], 0, [[16, NQ], [1, 16]]),
                    in_=v(cs[b], 15, [[CW, NQ], [16, 16], [15, 16]]),
                    axis=X,
                ).then_inc(redsem, 1)

            for b in range(B):
                m = nc.vector.memset(pps[b][0:NP, 0:1], 0.0)
                if b == B - 1:
                    m.then_inc(initsem, 1)
            scan(0)
            scan(1)
            scan(2)
            red(0)
            scan(3)
            red(1)
            red(2)
            red(3)

        @block.gpsimd
        def _(gpsimd):
            for b in range(B):
                # C[16t+m] = P[32t+m+16] - P[32t+m]  (local window sums)
                gpsimd.wait_ge(vscan, b + 1)
                if b == 0:
                    gpsimd.wait_ge(initsem, 1)
                nc.gpsimd.tensor_tensor(
                    out=v(cs[b], 0, [[CW, NP], [16, 16], [1, 16]]),
                    in0=v(pps[b], 16, [[PW, NP], [32, 16], [1, 16]]),
                    in1=v(pps[b], 0, [[PW, NP], [32, 16], [1, 16]]),
                    op=sub_op,
                ).then_inc(subsem, 1)

        @block.scalar
        def _(scalar):
            for b in range(B):
                # tail partition 126's band rows (rows 2016..2031)
                scalar.dma_start(
                    v(bts[b], NQ * PW + 1, [[PW, 1], [32, 16], [1, 32]]),
                    bass.AP(
                        x,
                        b * MAT_ELEMS + 2 + NQ * 16 * ROW_STRIDE,
                        [[16 * ROW_STRIDE, 1], [ROW_STRIDE, 16], [1, 32]],
                    ),
                ).then_inc(tsem[b], 16)

            def halo(b):
                # C[q, 256+f] = C[q+1, f], f in [0, 240)
                scalar.wait_ge(subsem, b + 1)
                scalar.dma_start(
                    v(cs[b], 256, [[CW, NQ], [1, 240]]),
                    v(cs[b], CW, [[CW, NQ], [1, 240]]),
                ).then_inc(hsem[b], 16)

            def mulout(b):
                scalar.wait_ge(redsem, b + 1)
                nc.scalar.mul(
                    v(yos[b], 0, [[16, NQ], [1, 16]]),
                    v(yvs[b], 0, [[16, NQ], [1, 16]]),
                    1.0 / 256.0,
                ).then_inc(scalesem, 1)
                scalar.wait_ge(scalesem, b + 1)
                scalar.dma_start(
                    bass.AP(y, b * ND, [[16, NQ], [1, 16]]),
                    v(yos[b], 0, [[16, NQ], [1, 16]]),
                ).then_inc(outsem, 16)

            halo(0)
            halo(1)
            mulout(0)
            halo(2)
            mulout(1)
            halo(3)
            mulout(2)
            mulout(3)

    nc.compile()
    return nc


def _get_compiled():
    global _COMPILED
    if _COMPILED is None:
        _COMPILED = _build()
    return _COMPILED


def kernel(x: np.ndarray) -> np.ndarray:
    global LAST_EXEC_TIME_NS, LAST_TRACE_DIR
    from concourse.bass_utils import run_bass_kernel_spmd

    x = np.ascontiguousarray(np.asarray(x), dtype=np.float32)
    assert x.shape == (B_FULL, MAT, MAT), x.shape

    nc = _get_compiled()
    in_maps = [
        {"x": x[i * B_PER_CORE : (i + 1) * B_PER_CORE]} for i in range(N_CORES)
    ]
    trace = bool(int(os.environ.get("KERNEL_TRACE", "0")))
    kwargs = {}
    if trace:
        # test-only: keep NTFF artifacts local instead of uploading
        from concourse import bass_utils as _bu
        import tempfile

        _bu.upload_artifacts = lambda tmpdir: tmpdir
        LAST_TRACE_DIR = tempfile.mkdtemp(prefix="ktrace_")
        kwargs["tmpdir"] = LAST_TRACE_DIR
    res = run_bass_kernel_spmd(
        nc, in_maps, core_ids=list(range(N_CORES)), trace=trace, **kwargs
    )
    LAST_EXEC_TIME_NS = res.exec_time_ns
    out = np.concatenate([res.results[i]["y"] for i in range(N_CORES)], axis=0)
    return out.astype(np.float32)


# revision 20
# speedup vs baseline: 3.5726x; 1.0263x over previous
"""Trainium2 Bass kernel for DiamondLayer.

Computes out[b, d] = mean(x[b, d:d+16, d+17:d+33]) for d in [0, 2016):
16x16 mean-pool windows sliding along the diagonal of each 2048x2048 matrix.

Sharding: pure data parallel over batch - 32 batches -> 8 cores x 4 batches.

Per-core kernel (raw bacc, no Tile). Partition p holds band rows
[16p, 16p+16), each row the 32 cols [r+2, r+34) (one 128B DMA run per
row; ~1/64 of the matrix). Per batch:

  band DMA (sync queue, 127 partitions x 16 rows x 128B)
  -> DVE prefix scan P over the flat band
  -> GpSimd sub C[16t+m] = P[32t+m+16] - P[32t+m]   (contiguous 256)
  -> S16 halo (scalar queue): C[q+1, 0:240] -> C[q, 256:496], so
     partition q holds window sums for all 31 rows its diamonds touch
     (C[16tau+m], tau in [0,31), is injective: 16u+15s collides only at
     |u-u'| = 15 which is out of range)
  -> DVE reduce out[16q+u] = sum_s C[q, 15+16u+15s]  (256 reads)
  -> ACT scale 1/256 -> out DMA (scalar queue).

vs the previous version: the halo moves 960B/partition of window sums
instead of 1924B of prefix values, there is no separate subtract of
halo'd prefixes, and band DMAs carry all 127 partitions in one shot.
Per-DMA semaphores; compute deps via counting sems.
"""

import os
import sys

import numpy as np

for _p in ("/opt/trn_rl_repo",):
    if _p not in sys.path:
        sys.path.insert(0, _p)

B_FULL = 32
N_CORES = 8
B_PER_CORE = B_FULL // N_CORES  # 4
MAT = 2048
ND = MAT - 32  # 2016
NQ = ND // 16  # 126  (diamond groups)
NP = NQ + 1  # 127  (partitions holding band rows)
ROW_STRIDE = MAT + 1  # 2049
MAT_ELEMS = MAT * MAT
PW = 528  # band / prefix cols per partition (needs 513)
CW = 512  # window-sum cols per partition (uses 496)

LAST_EXEC_TIME_NS = None
LAST_TRACE_DIR = None
_COMPILED = None


def _build():
    import concourse.bass as bass
    import concourse.bacc as bacc
    from concourse import mybir
    from contextlib import ExitStack

    f32 = mybir.dt.float32
    add = mybir.AluOpType.add
    sub_op = mybir.AluOpType.subtract
    bypass = mybir.AluOpType.bypass
    X = mybir.AxisListType.X

    nc = bacc.Bacc("TRN2", target_bir_lowering=False, debug=False)
    x = nc.dram_tensor("x", [B_PER_CORE, MAT, MAT], f32, kind="ExternalInput")
    y = nc.dram_tensor("y", [B_PER_CORE, ND], f32, kind="ExternalOutput")

    def v(t, off, pat):
        return bass.AP(t, off, pat)

    with ExitStack() as ctx:
        B = B_PER_CORE
        e = ctx.enter_context
        bts = [e(nc.sbuf_tensor(f"bt{i}", [NP, PW], f32)) for i in range(B)]
        pps = [e(nc.sbuf_tensor(f"pp{i}", [NP, PW], f32)) for i in range(B)]
        cs = [e(nc.sbuf_tensor(f"c{i}", [NP, CW], f32)) for i in range(B)]
        yvs = [e(nc.sbuf_tensor(f"yv{i}", [NQ, 16], f32)) for i in range(B)]
        yos = [e(nc.sbuf_tensor(f"yo{i}", [NQ, 16], f32)) for i in range(B)]
        bsem = [e(nc.semaphore(f"bsem{i}")) for i in range(B)]
        tsem = [e(nc.semaphore(f"tsem{i}")) for i in range(B)]
        hsem = [e(nc.semaphore(f"hsem{i}")) for i in range(B)]
        vscan = e(nc.semaphore("vscan"))
        initsem = e(nc.semaphore("initsem"))
        subsem = e(nc.semaphore("subsem"))
        sub3sem = e(nc.semaphore("sub3sem"))
        redsem = e(nc.semaphore("redsem"))
        scalesem = e(nc.semaphore("scalesem"))
        outsem = e(nc.semaphore("outsem"))
        block = e(nc.Block(no_gpsimd_drain=True))

        @block.sync
        def _(sync):
            for b in range(B):
                # band: bt[p, 1+32t+j] = x[b, 16p+t, 16p+t+2+j]
                # 126 partitions; 127 in one DMA lands all packets on a
                # single SDMA engine (odd partition count breaks the
                # HWDGE 16-way split) - tail partition goes separately.
                sync.dma_start(
                    v(bts[b], 1, [[PW, NQ], [32, 16], [1, 32]]),
                    bass.AP(
                        x,
                        b * MAT_ELEMS + 2,
                        [[16 * ROW_STRIDE, NQ], [ROW_STRIDE, 16], [1, 32]],
                    ),
                ).then_inc(bsem[b], 16)
            for b in range(B):
                sync.wait_ge(scalesem, b + 1)
                sync.dma_start(
                    bass.AP(y, b * ND, [[16, NQ], [1, 16]]),
                    v(yos[b], 0, [[16, NQ], [1, 16]]),
                ).then_inc(outsem, 16)
            sync.wait_ge(outsem, 16 * B)

        @block.vector
        def _(vector):
            def scan(b):
                vector.wait_ge(bsem[b], 16)
                vector.wait_ge(tsem[b], 16)
                # P[f] = prefix sum of the flat band per partition; P[0] = 0
                nc.vector.tensor_tensor_scan(
                    out=v(pps[b], 1, [[PW, NP], [1, 511]]),
                    data0=v(bts[b], 1, [[PW, NP], [1, 511]]),
                    data1=v(bts[b], 1, [[PW, NP], [1, 511]]),
                    initial=0.0,
                    op0=add,
                    op1=bypass,
                ).then_inc(vscan, 1)

            def red(b):
                # out[16q+u] = sum_s C[q, 15+16u+15s]
                vector.wait_ge(hsem[b], 16)
                nc.vector.reduce_sum(
                    out=v(yvs[b], 0, [[16, NQ], [1, 16]]),
                    in_=v(cs[b], 15, [[CW, NQ], [16, 16], [15, 16]]),
                    axis=X,
                ).then_inc(redsem, 1)
                # scale on DVE right after; the engine pipelines under
                # relaxed ordering, so even same-engine RAW needs a sem
                vector.wait_ge(redsem, b + 1)
                nc.vector.tensor_scalar_mul(
                    v(yos[b], 0, [[16, NQ], [1, 16]]),
                    v(yvs[b], 0, [[16, NQ], [1, 16]]),
                    1.0 / 256.0,
                ).then_inc(scalesem, 1)

            for b in range(B):
                m = nc.vector.memset(pps[b][0:NP, 0:1], 0.0)
                if b == B - 1:
                    m.then_inc(initsem, 1)
            scan(0)
            scan(1)
            scan(2)
            red(0)
            scan(3)
            # sub for the last batch inline on DVE: saves the gpsimd hop
            # on the critical chain (sem wait: see relaxed-ordering note)
            vector.wait_ge(vscan, 4)
            nc.vector.tensor_tensor(
                out=v(cs[3], 0, [[CW, NP], [16, 16], [1, 16]]),
                in0=v(pps[3], 16, [[PW, NP], [32, 16], [1, 16]]),
                in1=v(pps[3], 0, [[PW, NP], [32, 16], [1, 16]]),
                op=sub_op,
            ).then_inc(sub3sem, 1)
            red(1)
            red(2)
            red(3)

        @block.gpsimd
        def _(gpsimd):
            for b in range(B - 1):
                # C[16t+m] = P[32t+m+16] - P[32t+m]  (local window sums)
                gpsimd.wait_ge(vscan, b + 1)
                if b == 0:
                    gpsimd.wait_ge(initsem, 1)
                nc.gpsimd.tensor_tensor(
                    out=v(cs[b], 0, [[CW, NP], [16, 16], [1, 16]]),
                    in0=v(pps[b], 16, [[PW, NP], [32, 16], [1, 16]]),
                    in1=v(pps[b], 0, [[PW, NP], [32, 16], [1, 16]]),
                    op=sub_op,
                ).then_inc(subsem, 1)

        @block.scalar
        def _(scalar):
            for b in range(B):
                # tail partition 126's band rows (rows 2016..2031)
                scalar.dma_start(
                    v(bts[b], NQ * PW + 1, [[PW, 1], [32, 16], [1, 32]]),
                    bass.AP(
                        x,
                        b * MAT_ELEMS + 2 + NQ * 16 * ROW_STRIDE,
                        [[16 * ROW_STRIDE, 1], [ROW_STRIDE, 16], [1, 32]],
                    ),
                ).then_inc(tsem[b], 16)

            def halo(b):
                # C[q, 256+f] = C[q+1, f], f in [0, 240)
                if b == B - 1:
                    scalar.wait_ge(sub3sem, 1)
                else:
                    scalar.wait_ge(subsem, b + 1)
                scalar.dma_start(
                    v(cs[b], 256, [[CW, NQ], [1, 240]]),
                    v(cs[b], CW, [[CW, NQ], [1, 240]]),
                ).then_inc(hsem[b], 16)

            halo(0)
            halo(1)
            halo(2)
            halo(3)

    nc.compile()
    return nc


def _get_compiled():
    global _COMPILED
    if _COMPILED is None:
        _COMPILED = _build()
    return _COMPILED


def kernel(x: np.ndarray) -> np.ndarray:
    global LAST_EXEC_TIME_NS, LAST_TRACE_DIR
    from concourse.bass_utils import run_bass_kernel_spmd

    x = np.ascontiguousarray(np.asarray(x), dtype=np.float32)
    assert x.shape == (B_FULL, MAT, MAT), x.shape

    nc = _get_compiled()
    in_maps = [
        {"x": x[i * B_PER_CORE : (i + 1) * B_PER_CORE]} for i in range(N_CORES)
    ]
    trace = bool(int(os.environ.get("KERNEL_TRACE", "0")))
    kwargs = {}
    if trace:
        # test-only: keep NTFF artifacts local instead of uploading
        from concourse import bass_utils as _bu
        import tempfile

        _bu.upload_artifacts = lambda tmpdir: tmpdir
        LAST_TRACE_DIR = tempfile.mkdtemp(prefix="ktrace_")
        kwargs["tmpdir"] = LAST_TRACE_DIR
    res = run_bass_kernel_spmd(
        nc, in_maps, core_ids=list(range(N_CORES)), trace=trace, **kwargs
    )
    LAST_EXEC_TIME_NS = res.exec_time_ns
    out = np.concatenate([res.results[i]["y"] for i in range(N_CORES)], axis=0)
    return out.astype(np.float32)


# revision 21
# speedup vs baseline: 3.6069x; 1.0096x over previous
"""Trainium2 Bass kernel for DiamondLayer.

Computes out[b, d] = mean(x[b, d:d+16, d+17:d+33]) for d in [0, 2016):
16x16 mean-pool windows sliding along the diagonal of each 2048x2048 matrix.

Sharding: pure data parallel over batch - 32 batches -> 8 cores x 4 batches.

Per-core kernel (raw bacc, no Tile). Partition p holds band rows
[16p, 16p+16), each row the 32 cols [r+2, r+34) (one 128B DMA run per
row; ~1/64 of the matrix). Per batch:

  band DMA (sync queue, 127 partitions x 16 rows x 128B)
  -> DVE prefix scan P over the flat band
  -> GpSimd sub C[16t+m] = P[32t+m+16] - P[32t+m]   (contiguous 256)
  -> S16 halo (scalar queue): C[q+1, 0:240] -> C[q, 256:496], so
     partition q holds window sums for all 31 rows its diamonds touch
     (C[16tau+m], tau in [0,31), is injective: 16u+15s collides only at
     |u-u'| = 15 which is out of range)
  -> DVE reduce out[16q+u] = sum_s C[q, 15+16u+15s]  (256 reads)
  -> ACT scale 1/256 -> out DMA (scalar queue).

vs the previous version: the halo moves 960B/partition of window sums
instead of 1924B of prefix values, there is no separate subtract of
halo'd prefixes, and band DMAs carry all 127 partitions in one shot.
Per-DMA semaphores; compute deps via counting sems.
"""

import os
import sys

import numpy as np

for _p in ("/opt/trn_rl_repo",):
    if _p not in sys.path:
        sys.path.insert(0, _p)

B_FULL = 32
N_CORES = 8
B_PER_CORE = B_FULL // N_CORES  # 4
MAT = 2048
ND = MAT - 32  # 2016
NQ = ND // 16  # 126  (diamond groups)
NP = NQ + 1  # 127  (partitions holding band rows)
ROW_STRIDE = MAT + 1  # 2049
MAT_ELEMS = MAT * MAT
PW = 528  # band / prefix cols per partition (needs 513)
CW = 512  # window-sum cols per partition (uses 496)

LAST_EXEC_TIME_NS = None
LAST_TRACE_DIR = None
_COMPILED = None


def _build():
    import concourse.bass as bass
    import concourse.bacc as bacc
    from concourse import mybir
    from contextlib import ExitStack

    f32 = mybir.dt.float32
    add = mybir.AluOpType.add
    sub_op = mybir.AluOpType.subtract
    bypass = mybir.AluOpType.bypass
    X = mybir.AxisListType.X

    nc = bacc.Bacc("TRN2", target_bir_lowering=False, debug=False)
    x = nc.dram_tensor("x", [B_PER_CORE, MAT, MAT], f32, kind="ExternalInput")
    y = nc.dram_tensor("y", [B_PER_CORE, ND], f32, kind="ExternalOutput")

    def v(t, off, pat):
        return bass.AP(t, off, pat)

    with ExitStack() as ctx:
        B = B_PER_CORE
        e = ctx.enter_context
        bts = [e(nc.sbuf_tensor(f"bt{i}", [NP, PW], f32)) for i in range(B)]
        pps = [e(nc.sbuf_tensor(f"pp{i}", [NP, PW], f32)) for i in range(B)]
        cs = [e(nc.sbuf_tensor(f"c{i}", [NP, CW], f32)) for i in range(B)]
        yvs = [e(nc.sbuf_tensor(f"yv{i}", [NQ, 16], f32)) for i in range(B)]
        yos = [e(nc.sbuf_tensor(f"yo{i}", [NQ, 16], f32)) for i in range(B)]
        bsemA = [e(nc.semaphore(f"bsa{i}")) for i in range(B)]
        bsemB = [e(nc.semaphore(f"bsb{i}")) for i in range(B)]
        tsem = [e(nc.semaphore(f"tsem{i}")) for i in range(B)]
        hsem = [e(nc.semaphore(f"hsem{i}")) for i in range(B)]
        vscanA = e(nc.semaphore("vscanA"))
        vscanB = e(nc.semaphore("vscanB"))
        initsem = e(nc.semaphore("initsem"))
        subAsem = e(nc.semaphore("subAsem"))
        subBsem = e(nc.semaphore("subBsem"))
        sub3asem = e(nc.semaphore("sub3asem"))
        sub3bsem = e(nc.semaphore("sub3bsem"))
        redsem = e(nc.semaphore("redsem"))
        scalesem = e(nc.semaphore("scalesem"))
        outsem = e(nc.semaphore("outsem"))
        block = e(nc.Block(no_gpsimd_drain=True))

        @block.sync
        def _(sync):
            # Per batch: A-part = band rows 0..14 of each partition (all
            # the halo depends on), B-part = row 15. The halo for batch b
            # can then fly while later batches' bands still drain.
            # 126 partitions per DMA; 127 in one DMA lands every packet
            # on a single SDMA engine (odd partition count breaks the
            # HWDGE 16-way split) - tail partition goes via scalar queue.
            for b in range(B):
                sync.dma_start(
                    v(bts[b], 1, [[PW, NQ], [32, 15], [1, 32]]),
                    bass.AP(
                        x,
                        b * MAT_ELEMS + 2,
                        [[16 * ROW_STRIDE, NQ], [ROW_STRIDE, 15], [1, 32]],
                    ),
                ).then_inc(bsemA[b], 16)
                sync.dma_start(
                    v(bts[b], 481, [[PW, NQ], [1, 32]]),
                    bass.AP(
                        x,
                        b * MAT_ELEMS + 2 + 15 * ROW_STRIDE,
                        [[16 * ROW_STRIDE, NQ], [1, 32]],
                    ),
                ).then_inc(bsemB[b], 16)
            for b in range(B):
                sync.wait_ge(scalesem, b + 1)
                sync.dma_start(
                    bass.AP(y, b * ND, [[16, NQ], [1, 16]]),
                    v(yos[b], 0, [[16, NQ], [1, 16]]),
                ).then_inc(outsem, 16)
            sync.wait_ge(outsem, 16 * B)

        @block.vector
        def _(vector):
            def scanA(b):
                # P[f] = prefix of band rows 0..14; writes pp[1..480),
                # pp[0] and pp[480] stay memset-0
                vector.wait_ge(bsemA[b], 16)
                vector.wait_ge(tsem[b], 16)
                nc.vector.tensor_tensor_scan(
                    out=v(pps[b], 1, [[PW, NP], [1, 479]]),
                    data0=v(bts[b], 1, [[PW, NP], [1, 479]]),
                    data1=v(bts[b], 1, [[PW, NP], [1, 479]]),
                    initial=0.0,
                    op0=add,
                    op1=bypass,
                ).then_inc(vscanA, 1)

            def scanB(b):
                # independent prefix of band row 15 into pp[481..513)
                vector.wait_ge(bsemB[b], 16)
                vector.wait_ge(tsem[b], 16)
                nc.vector.tensor_tensor_scan(
                    out=v(pps[b], 481, [[PW, NP], [1, 32]]),
                    data0=v(bts[b], 481, [[PW, NP], [1, 32]]),
                    data1=v(bts[b], 481, [[PW, NP], [1, 32]]),
                    initial=0.0,
                    op0=add,
                    op1=bypass,
                ).then_inc(vscanB, 1)

            def red(b):
                # out[16q+u] = sum_s C[q, 15+16u+15s]
                vector.wait_ge(hsem[b], 16)
                if b == B - 1:
                    vector.wait_ge(sub3bsem, 1)
                else:
                    vector.wait_ge(subBsem, b + 1)
                nc.vector.reduce_sum(
                    out=v(yvs[b], 0, [[16, NQ], [1, 16]]),
                    in_=v(cs[b], 15, [[CW, NQ], [16, 16], [15, 16]]),
                    axis=X,
                ).then_inc(redsem, 1)
                # scale on DVE right after; the engine pipelines under
                # relaxed ordering, so even same-engine RAW needs a sem
                vector.wait_ge(redsem, b + 1)
                nc.vector.tensor_scalar_mul(
                    v(yos[b], 0, [[16, NQ], [1, 16]]),
                    v(yvs[b], 0, [[16, NQ], [1, 16]]),
                    1.0 / 256.0,
                ).then_inc(scalesem, 1)

            for b in range(B):
                nc.vector.memset(pps[b][0:NP, 0:1], 0.0)
                m = nc.vector.memset(
                    v(pps[b], 480, [[PW, NP], [1, 1]]), 0.0
                )
                if b == B - 1:
                    m.then_inc(initsem, 1)
            scanA(0)
            scanB(0)
            scanA(1)
            scanB(1)
            scanA(2)
            scanB(2)
            scanA(3)
            scanB(3)
            # batch 3 subs inline on DVE: saves the gpsimd hop on the
            # critical chain (sem waits: relaxed-ordering RAW rule)
            vector.wait_ge(vscanA, 4)
            nc.vector.tensor_tensor(
                out=v(cs[3], 0, [[CW, NP], [16, 15], [1, 16]]),
                in0=v(pps[3], 16, [[PW, NP], [32, 15], [1, 16]]),
                in1=v(pps[3], 0, [[PW, NP], [32, 15], [1, 16]]),
                op=sub_op,
            ).then_inc(sub3asem, 1)
            vector.wait_ge(vscanB, 4)
            nc.vector.tensor_tensor(
                out=v(cs[3], 240, [[CW, NP], [1, 16]]),
                in0=v(pps[3], 496, [[PW, NP], [1, 16]]),
                in1=v(pps[3], 480, [[PW, NP], [1, 16]]),
                op=sub_op,
            ).then_inc(sub3bsem, 1)
            red(0)
            red(1)
            red(2)
            red(3)

        @block.gpsimd
        def _(gpsimd):
            for b in range(B - 1):
                # C[16t+m] = P[32t+m+16] - P[32t+m], rows 0..14
                gpsimd.wait_ge(vscanA, b + 1)
                if b == 0:
                    gpsimd.wait_ge(initsem, 1)
                nc.gpsimd.tensor_tensor(
                    out=v(cs[b], 0, [[CW, NP], [16, 15], [1, 16]]),
                    in0=v(pps[b], 16, [[PW, NP], [32, 15], [1, 16]]),
                    in1=v(pps[b], 0, [[PW, NP], [32, 15], [1, 16]]),
                    op=sub_op,
                ).then_inc(subAsem, 1)
                # row 15 windows from the independent row-15 prefix
                gpsimd.wait_ge(vscanB, b + 1)
                nc.gpsimd.tensor_tensor(
                    out=v(cs[b], 240, [[CW, NP], [1, 16]]),
                    in0=v(pps[b], 496, [[PW, NP], [1, 16]]),
                    in1=v(pps[b], 480, [[PW, NP], [1, 16]]),
                    op=sub_op,
                ).then_inc(subBsem, 1)

        @block.scalar
        def _(scalar):
            for b in range(B):
                # tail partition 126's band rows (rows 2016..2031)
                scalar.dma_start(
                    v(bts[b], NQ * PW + 1, [[PW, 1], [32, 16], [1, 32]]),
                    bass.AP(
                        x,
                        b * MAT_ELEMS + 2 + NQ * 16 * ROW_STRIDE,
                        [[16 * ROW_STRIDE, 1], [ROW_STRIDE, 16], [1, 32]],
                    ),
                ).then_inc(tsem[b], 16)

            def halo(b):
                # C[q, 256+f] = C[q+1, f], f in [0, 240): only rows 0..14
                # of the neighbor are ever needed, i.e. subA alone
                if b == B - 1:
                    scalar.wait_ge(sub3asem, 1)
                else:
                    scalar.wait_ge(subAsem, b + 1)
                scalar.dma_start(
                    v(cs[b], 256, [[CW, NQ], [1, 240]]),
                    v(cs[b], CW, [[CW, NQ], [1, 240]]),
                ).then_inc(hsem[b], 16)

            halo(0)
            halo(1)
            halo(2)
            halo(3)

    nc.compile()
    return nc


def _get_compiled():
    global _COMPILED
    if _COMPILED is None:
        _COMPILED = _build()
    return _COMPILED


def kernel(x: np.ndarray) -> np.ndarray:
    global LAST_EXEC_TIME_NS, LAST_TRACE_DIR
    from concourse.bass_utils import run_bass_kernel_spmd

    x = np.ascontiguousarray(np.asarray(x), dtype=np.float32)
    assert x.shape == (B_FULL, MAT, MAT), x.shape

    nc = _get_compiled()
    in_maps = [
        {"x": x[i * B_PER_CORE : (i + 1) * B_PER_CORE]} for i in range(N_CORES)
    ]
    trace = bool(int(os.environ.get("KERNEL_TRACE", "0")))
    kwargs = {}
    if trace:
        # test-only: keep NTFF artifacts local instead of uploading
        from concourse import bass_utils as _bu
        import tempfile

        _bu.upload_artifacts = lambda tmpdir: tmpdir
        LAST_TRACE_DIR = tempfile.mkdtemp(prefix="ktrace_")
        kwargs["tmpdir"] = LAST_TRACE_DIR
    res = run_bass_kernel_spmd(
        nc, in_maps, core_ids=list(range(N_CORES)), trace=trace, **kwargs
    )
    LAST_EXEC_TIME_NS = res.exec_time_ns
    out = np.concatenate([res.results[i]["y"] for i in range(N_CORES)], axis=0)
    return out.astype(np.float32)
